# revision 13
# baseline (speedup 1.0000x reference)
"""Trainium2 Bass kernel for a 2-layer bidirectional LSTM char model (B=32,
T=1024, EMB=128, HID=256, OUT=5).

kernel(**inputs) takes the FULL unsharded inputs, returns FULL [B,T,5] f32
logits. Data-parallel over batch on 8 NeuronCores (4 examples/core); each
core runs all four scans (2 layers x 2 dirs), fw/bw interleaved per step so
engines pipeline across the two independent chains.

v2 redesign — minimize the per-step critical path (wall = 2048 sequential
cell steps x step latency):
  - xp (input projection + bias) is injected into the z PSUM tile via an
    identity matmul at the head of the accumulation group; it has no h
    dependency so it runs during the h-wait, and the PSUM-resident z means
    no zs-add on the critical path.
  - ONE sigmoid activation covers all four gates: the j-gate's weights,
    bias and xp are pre-doubled host-side so sigmoid(2*z_j) gives
    tanh(z_j) = 2*sigmoid(2*z_j) - 1, recovered inside the DVE chain.
  - c-update in 3 DVE ops using scalar_tensor_tensor:
        t  = (s_j - 0.5) * s_i          # = u/2
        c1 = s_f * c_prev
        c  = 2*t + c1
  - bw sequence masking via xp POISONING instead of per-step mask muls:
    for masked positions (t >= len), xp is overwritten so z_i = -30 and
    z_f = +30, making c (and hence h) carry through unchanged (state stays
    exactly 0 through the masked prefix of the bw scan). No mask ops on the
    recurrent chain at all.
  - recurrent weights stored fp8 e3m4 (x16 scaled; descaled for free via
    the sigmoid's scale operand) => 4x faster LDWEIGHTS (FWL) and a shorter
    post-h PE tail. Set WDT=bf16 to fall back.
  - xp scratch in DRAM as bf16 (halved traffic), scaled x16 to match.

Layouts as baseline: units on partitions; hist[l][d]: [128, (T+1)*8] bf16,
col = slot*8 + khalf*4 + b; fw writes slot t+1/reads t; bw writes t/reads
t+1; logits phase masks concat(fw1,bw1) then out_W matmul.
"""

import os
import numpy as np
import ml_dtypes

B, VOCAB, EMB, HID, OUT = 32, 256, 128, 256, 5
T_FULL = 1024
FORGET_BIAS = 1.0
NCORES = 8
BL = B // NCORES  # 4
CH = 64
POISON = 30.0

bf16 = ml_dtypes.bfloat16
_cache = {}


def _wdt():
    return os.environ.get("WDT", "bf16")


def _wscale():
    return 16.0 if _wdt() == "fp8" else 1.0


def _tile_lhsT(W, nk, nm):
    """[K=nk*128, M=nm*128] -> [128, nk*nm*128], col block (k*nm+m)."""
    return np.ascontiguousarray(
        W.reshape(nk, 128, nm, 128).transpose(1, 0, 2, 3).reshape(128, nk * nm * 128)
    )


def _patch_tile_drain(tile_mod, mybir):
    """Pinned walrus rejects >1 sync wait on a Drain; split extras onto NOPs."""
    if getattr(tile_mod, "_drain_patched", False):
        return

    def _drain_and_barrier(self, tick_clock, wait_clock):
        nc = self.nc
        drain_inst = nc.sync.drain()
        wait_clock.add_sem_waits(
            drain_inst.ins, tile_mod.ScopedClock({None: tick_clock.global_clock})
        )
        si = drain_inst.ins.sync_info
        if si is not None and len(si.on_wait) > 1:
            waits = list(si.on_wait)
            drain_inst.ins.sync_info = mybir.SyncInfo(
                on_wait=waits[:1], on_update=list(si.on_update)
            )
            for w in waits[1:]:
                nop = nc.sync.nop(nofuse=True, hint="drain_wait_split")
                nop.ins.sync_info = mybir.SyncInfo(on_wait=[w], on_update=[])
        nc.all_engine_barrier()
        assert self.sems is not None
        popped = nc._tile_sem_poison_stack.pop()
        assert popped is self._sem_poison
        nc.clear_and_free_semaphores(list(self.sems.allocated().values()))
        nc.all_engine_barrier()

    tile_mod.TileContext._drain_and_barrier = _drain_and_barrier
    tile_mod._drain_patched = True


def _patch_compiler_wait_split():
    """Pinned walrus accepts only 1 sync wait per instruction encoding slot
    it has available; rewrite the BIR before compiling so every instruction
    carries at most 1 wait, extras moved to preceding same-engine NoOps."""
    import json
    import concourse.bass_utils as bu
    import concourse.bass2jax as b2j

    if getattr(bu, "_wsplit_patched", False):
        return
    orig = bu.compile_bir_kernel

    def fix_block(bb, ctr):
        out = []
        for inst in bb.get("instructions", []):
            for blk in inst.get("blocks") or []:
                fix_block(blk, ctr)
            si = inst.get("sync_info")
            if si:
                ow = si.get("on_wait") or []
                if len(ow) > 1:
                    for w in ow[:-1]:
                        ctr[0] += 1
                        out.append(
                            {
                                "debug": inst.get("debug", 0),
                                "engine": inst["engine"],
                                "ins": [],
                                "name": f"wsplit-{ctr[0]}",
                                "opcode": "NoOp",
                                "outs": [],
                                "text_hint": "wsplit",
                                "sync_info": {"on_wait": [w], "on_update": []},
                            }
                        )
                    si["on_wait"] = [ow[-1]]
            out.append(inst)
        bb["instructions"] = out

    def wrapped(bir_json, tmpdir, neff_name="file.neff"):
        b = json.loads(bir_json)
        ctr = [0]
        for f in b.get("functions", []):
            for bb in f.get("blocks", []):
                fix_block(bb, ctr)
        return orig(json.dumps(b).encode(), tmpdir, neff_name)

    bu.compile_bir_kernel = wrapped
    b2j.compile_bir_kernel = wrapped

    if os.environ.get("LDW_OPT", "0") == "1":
        orig_run = bu.run_command

        def run_patched(argv, **kw):
            argv = [
                "--enable-ldw-opt=true" if a == "--enable-ldw-opt=false" else a
                for a in argv
            ]
            return orig_run(argv, **kw)

        bu.run_command = run_patched
    bu._wsplit_patched = True


def _build(T):
    import concourse.bass as bass
    import concourse.mybir as mybir
    import concourse.tile as tile

    _patch_tile_drain(tile, mybir)
    _patch_compiler_wait_split()
    f32 = mybir.dt.float32
    b16 = mybir.dt.bfloat16
    f16 = mybir.dt.float16
    w8 = mybir.dt.float8e3
    wdt = w8 if _wdt() == "fp8" else b16
    wnp_bytes = 1 if _wdt() == "fp8" else 2
    Sig = mybir.ActivationFunctionType.Sigmoid
    Tanh = mybir.ActivationFunctionType.Tanh
    ADD = mybir.AluOpType.add
    MUL = mybir.AluOpType.mult
    NT = T * BL
    WS = _wscale()

    nc = bass.Bass("TRN2", target_bir_lowering=False)

    x0T = nc.dram_tensor("x0T", [128, NT], b16, kind="ExternalInput")
    mask8 = nc.dram_tensor("mask8", [128, 8 * T], b16, kind="ExternalInput")
    whs, wxs, biases = {}, {}, {}
    for l in range(2):
        nk = 1 if l == 0 else 4
        for d in ("f", "b"):
            whs[(l, d)] = nc.dram_tensor(f"wh{l}{d}", [128, 16 * 128], wdt, kind="ExternalInput")
            wxs[(l, d)] = nc.dram_tensor(f"wx{l}{d}", [128, nk * 8 * 128], b16, kind="ExternalInput")
            biases[(l, d)] = nc.dram_tensor(f"bias{l}{d}", [128, 8], f32, kind="ExternalInput")
    outw = nc.dram_tensor("outw", [128, 20], b16, kind="ExternalInput")
    outb = nc.dram_tensor("outb", [128, 1], f32, kind="ExternalInput")
    identd = nc.dram_tensor("ident", [128, 128], f16, kind="ExternalInput")
    out = nc.dram_tensor("out", [5, NT], f32, kind="ExternalOutput")

    with tile.TileContext(nc) as tc:
        with tc.tile_pool(name="persist", bufs=1) as pp, \
             tc.tile_pool(name="xpbuf", bufs=2) as xpp, \
             tc.tile_pool(name="stage", bufs=3) as sp, \
             tc.tile_pool(name="small", bufs=2) as mp, \
             tc.tile_pool(name="zps", bufs=2, space="PSUM") as zp, \
             tc.tile_pool(name="pps", bufs=2, space="PSUM") as qp, \
             tc.tile_pool(name="xpd", bufs=1, space="DRAM") as dp:

            def load(name, dram, shape, dt):
                t = pp.tile(shape, dt, tag=name, name=name)
                nc.sync.dma_start(t[:], dram[:])
                return t

            x0T_s = load("x0T", x0T, [128, NT], b16)
            mask_s = load("mask8", mask8, [128, 8 * T], b16)
            wh_s = {k: load(f"wh{k[0]}{k[1]}", v, [128, 16 * 128], wdt) for k, v in whs.items()}
            wx_s = {k: load(f"wx{k[0]}{k[1]}", v, list(v.shape), b16) for k, v in wxs.items()}
            bias_s = {k: load(f"bias{k[0]}{k[1]}", v, [128, 8], f32) for k, v in biases.items()}
            outw_s = load("outw", outw, [128, 20], b16)
            outb_s = load("outb", outb, [128, 1], f32)

            # fp16 identity for the xp->PSUM inject matmul
            ident = load("ident", identd, [128, 128], f16)

            hist = {}
            for l in range(2):
                for d in ("f", "b"):
                    hist[(l, d)] = pp.tile([128, (T + 1) * 8], b16, tag=f"hist{l}{d}", name=f"hist{l}{d}")
            for l in range(2):
                nc.vector.memset(hist[(l, "f")][:, 0:8], 0.0)
                nc.vector.memset(hist[(l, "b")][:, T * 8 : T * 8 + 8], 0.0)

            xp_dram = {
                (l, d): dp.tile([128, T * 32], f16, tag=f"xp{l}{d}", name=f"xp{l}{d}")
                for l in range(2)
                for d in ("f", "b")
            }

            def hist_rhs(l, d, half, c0):
                """[128, 128t, 4b] output slice of a history for XP1/logits.
                fw output for position t is slot t+1; bw output is slot t."""
                r = hist[(l, d)].rearrange("p (s q) -> p s q", q=8)
                s0 = c0 + 1 if d == "f" else c0
                return r[:, s0 : s0 + 128, half * 4 : half * 4 + 4]

            # poison per m-slot (gate order i,i,j,j -> wait: TF order i,j,f,o
            # => m-slots 0,1=i  2,3=j  4,5=f  6,7=o). Only i and f need
            # poisoning: i -> -POISON (u term ~ 0), f -> +POISON (c carries).
            PVAL = {0: -POISON * WS, 1: -POISON * WS, 4: POISON * WS, 5: POISON * WS}

            def xp_phase(l, d):
                nk = 1 if l == 0 else 4
                wx_t = wx_s[(l, d)]
                xp_r = xp_dram[(l, d)].rearrange("p (s q) -> p s q", q=32)
                mask_r = mask_s.rearrange("p (s q) -> p s q", q=8)
                for c0 in range(0, T, 128):
                    for m in range(8):
                        ps = qp.tile([128, 512], f32, tag="proj_ps")
                        for k in range(nk):
                            if l == 0:
                                rhs = x0T_s[:, c0 * 4 : c0 * 4 + 512]
                            else:
                                rhs = hist_rhs(0, "f" if k < 2 else "b", k % 2, c0)
                            nc.tensor.matmul(
                                ps[:],
                                wx_t[:, (k * 8 + m) * 128 : (k * 8 + m + 1) * 128],
                                rhs,
                                start=(k == 0),
                                stop=(k == nk - 1),
                            )
                        st = sp.tile([128, 512], f16, tag="xp_st")
                        if d == "b" and m in PVAL:
                            p = PVAL[m]
                            # xp' = (ps + bias - p) * mask + p
                            s1 = sp.tile([128, 512], f32, tag="xp_s1")
                            nc.vector.tensor_scalar_add(s1[:], ps[:], biasP_s[(l, d)][:, m : m + 1])
                            mview = mask_r[:, c0 : c0 + 128, 0:4]
                            nc.vector.tensor_tensor(
                                st.rearrange("p (s q) -> p s q", q=4)[:],
                                s1.rearrange("p (s q) -> p s q", q=4)[:],
                                mview, MUL,
                            )
                            nc.vector.tensor_scalar_add(st[:], st[:], float(p))
                        else:
                            nc.vector.tensor_scalar_add(st[:], ps[:], bias_s[(l, d)][:, m : m + 1])
                        nc.sync.dma_start(xp_r[:, c0 : c0 + 128, m * 4 : m * 4 + 4], st[:])

            # bias - poison tiles for the poisoned m-slots (per l, bw only)
            biasP_s = {}
            for l in range(2):
                biasP_s[(l, "b")] = pp.tile([128, 8], f32, tag=f"biasP{l}", name=f"biasP{l}")
                bb = biasP_s[(l, "b")]
                nc.vector.tensor_copy(bb[:], bias_s[(l, "b")][:])
                for m, p in PVAL.items():
                    nc.vector.tensor_scalar_add(bb[:, m : m + 1], bb[:, m : m + 1], float(-p))

            def step(l, d, t, xp_tile, i):
                """One LSTM step: z = wh.T@h + xp (PSUM), s = sigmoid(z/WS),
                c = s_f*c + s_i*(2*s_j-1), h = s_o*tanh(c)."""
                h = hist[(l, d)]
                if d == "f":
                    r_off, w_off = t * 8, (t + 1) * 8
                else:
                    r_off, w_off = (t + 1) * 8, t * 8
                z = zp.tile([128, 32], f32, tag=f"z{d}")
                nc.tensor.matmul(
                    z[:], ident[:], xp_tile[:, i * 32 : i * 32 + 32],
                    start=True, stop=False, skip_group_check=True,
                )
                wh_t = wh_s[(l, d)]
                for m in range(8):
                    for k in range(2):
                        nc.tensor.matmul(
                            z[:, 4 * m : 4 * m + 4],
                            wh_t[:, (k * 8 + m) * 128 : (k * 8 + m + 1) * 128],
                            h[:, r_off + 4 * k : r_off + 4 * k + 4],
                            start=False,
                            stop=(m == 7 and k == 1),
                            skip_group_check=True,
                        )
                s = sp.tile([128, 32], f32, tag=f"s{d}")
                nc.scalar.activation(s[:], z[:], Sig, scale=1.0 / WS)
                c = cs[(l, d)]
                tt = mp.tile([128, 8], f32, tag=f"tt{d}")
                # tt = (s_j - 0.5) * s_i
                nc.vector.scalar_tensor_tensor(tt[:], s[:, 8:16], -0.5, s[:, 0:8], ADD, MUL)
                c1 = mp.tile([128, 8], f32, tag=f"c1{d}")
                nc.vector.tensor_tensor(c1[:], s[:, 16:24], c[:], MUL)
                # c = 2*tt + c1
                nc.vector.scalar_tensor_tensor(c[:], tt[:], 2.0, c1[:], MUL, ADD)
                th = mp.tile([128, 8], f32, tag=f"th{d}")
                nc.scalar.activation(th[:], c[:], Tanh)
                nc.vector.tensor_tensor(h[:, w_off : w_off + 8], s[:, 24:32], th[:], MUL)

            cs = {}
            for l in range(2):
                for d in ("f", "b"):
                    cs[(l, d)] = pp.tile([128, 8], f32, tag=f"c{l}{d}", name=f"c{l}{d}")

            for l in range(2):
                xp_phase(l, "f")
                xp_phase(l, "b")
                nc.vector.memset(cs[(l, "f")][:], 0.0)
                nc.vector.memset(cs[(l, "b")][:], 0.0)
                for c0 in range(0, T, CH):
                    xf = xpp.tile([128, CH * 32], f16, tag="xf")
                    nc.sync.dma_start(
                        xf[:], xp_dram[(l, "f")][:, c0 * 32 : (c0 + CH) * 32]
                    )
                    xb = xpp.tile([128, CH * 32], f16, tag="xb")
                    bw_lo = T - c0 - CH
                    nc.sync.dma_start(
                        xb[:], xp_dram[(l, "b")][:, bw_lo * 32 : (bw_lo + CH) * 32]
                    )
                    for i in range(CH):
                        step(l, "f", c0 + i, xf, i)
                        step(l, "b", T - 1 - (c0 + i), xb, CH - 1 - i)

            # ---- logits ----
            mask_r = mask_s.rearrange("p (s q) -> p s q", q=8)
            for c0 in range(0, T, 128):
                ps = qp.tile([128, 512], f32, tag="proj_ps")
                for k in range(4):
                    rhs = hist_rhs(1, "f" if k < 2 else "b", k % 2, c0)
                    nc.tensor.matmul(
                        ps[:5, :],
                        outw_s[:, k * 5 : k * 5 + 5],
                        rhs,
                        start=(k == 0),
                        stop=(k == 3),
                    )
                lg = sp.tile([5, 512], f32, tag="lg")
                nc.vector.tensor_mul(lg[:], ps[:5, :], mask_r[:5, c0 : c0 + 128, 0:4])
                nc.vector.tensor_scalar_add(lg[:], lg[:], outb_s[:5, 0:1])
                nc.sync.dma_start(out[:, c0 * 4 : c0 * 4 + 512], lg[:])

    return nc


last_results = None


def kernel(**inputs):
    global last_results
    T = int(os.environ.get("KERNEL_T", T_FULL))
    from concourse.bass_utils import run_bass_kernel_spmd

    tokens = np.asarray(inputs["tokens"])[:, :T]
    lengths = np.clip(np.asarray(inputs["lengths"]), 0, T)
    emb = np.asarray(inputs["emb"], dtype=np.float32)
    WS = _wscale()
    w8np = ml_dtypes.float8_e3m4

    if T not in _cache:
        _cache[T] = _build(T)
    nc = _cache[T]

    # ---- host-side retiling (shared across cores) ----
    shared = {}
    for l in range(2):
        D = EMB if l == 0 else 2 * HID
        nk = D // 128
        for d, pre in (("f", "fw"), ("b", "bw")):
            W = np.asarray(inputs[f"{pre}_W{l}"], dtype=np.float32).copy()
            bias = np.asarray(inputs[f"{pre}_b{l}"], dtype=np.float32).copy()
            bias[2 * HID : 3 * HID] += FORGET_BIAS
            # double the j-gate columns: sigmoid(2*z_j) = (tanh(z_j)+1)/2
            W[:, HID : 2 * HID] *= 2.0
            bias[HID : 2 * HID] *= 2.0
            wh = _tile_lhsT(W[D:] * WS, 2, 8)
            if _wdt() == "fp8":
                shared[f"wh{l}{d}"] = wh.astype(w8np)
            else:
                shared[f"wh{l}{d}"] = wh.astype(bf16)
            shared[f"wx{l}{d}"] = _tile_lhsT(W[:D] * WS, nk, 8).astype(bf16)
            shared[f"bias{l}{d}"] = np.ascontiguousarray(
                (bias * WS).reshape(8, 128).T
            ).astype(np.float32)
    shared["outw"] = np.ascontiguousarray(
        np.asarray(inputs["out_W"], dtype=np.float32)
        .reshape(4, 128, 5)
        .transpose(1, 0, 2)
        .reshape(128, 20)
    ).astype(bf16)
    ob = np.zeros((128, 1), np.float32)
    ob[:5, 0] = np.asarray(inputs["out_b"], dtype=np.float32)
    shared["outb"] = ob
    shared["ident"] = np.eye(128, dtype=np.float16)

    in_maps = []
    for ci in range(NCORES):
        bs = slice(ci * BL, (ci + 1) * BL)
        x0 = emb[tokens[bs]]  # [BL, T, 128]
        x0T = np.ascontiguousarray(x0.transpose(2, 1, 0).reshape(128, T * BL)).astype(bf16)
        mvec = (np.arange(T)[:, None] < lengths[bs][None, :]).astype(bf16)  # [T, BL]
        m8 = np.repeat(mvec[:, None, :], 2, axis=1).reshape(1, T * 8)
        mask8 = np.ascontiguousarray(np.broadcast_to(m8, (128, T * 8)))
        im = dict(shared)
        im["x0T"] = x0T
        im["mask8"] = mask8
        in_maps.append(im)

    res = run_bass_kernel_spmd(nc, in_maps, core_ids=list(range(NCORES)))
    last_results = res
    outs = []
    for ci in range(NCORES):
        o = res.results[ci]["out"]  # [5, T*BL]
        outs.append(o.reshape(5, T, BL).transpose(2, 1, 0))  # [BL, T, 5]
    return np.concatenate(outs, axis=0).astype(np.float32)


# revision 19
# speedup vs baseline: 3.0722x; 3.0722x over previous
"""Trainium2 Bass kernel for a 2-layer bidirectional LSTM char model (B=32,
T=1024, EMB=128, HID=256, OUT=5).

kernel(**inputs) takes the FULL unsharded inputs, returns FULL [B,T,5] f32
logits. Data-parallel over batch on 8 NeuronCores (4 examples/core); each
core runs all four scans (2 layers x 2 dirs), fw/bw interleaved per step so
engines pipeline across the two independent chains.

v2 redesign — minimize the per-step critical path (wall = 2048 sequential
cell steps x step latency):
  - xp (input projection + bias) is injected into the z PSUM tile via an
    identity matmul at the head of the accumulation group; it has no h
    dependency so it runs during the h-wait, and the PSUM-resident z means
    no zs-add on the critical path.
  - ONE sigmoid activation covers all four gates: the j-gate's weights,
    bias and xp are pre-doubled host-side so sigmoid(2*z_j) gives
    tanh(z_j) = 2*sigmoid(2*z_j) - 1, recovered inside the DVE chain.
  - c-update in 3 DVE ops using scalar_tensor_tensor:
        t  = (s_j - 0.5) * s_i          # = u/2
        c1 = s_f * c_prev
        c  = 2*t + c1
  - bw sequence masking via xp POISONING instead of per-step mask muls:
    for masked positions (t >= len), xp is overwritten so z_i = -30 and
    z_f = +30, making c (and hence h) carry through unchanged (state stays
    exactly 0 through the masked prefix of the bw scan). No mask ops on the
    recurrent chain at all.
  - recurrent weights stored fp8 e3m4 (x16 scaled; descaled for free via
    the sigmoid's scale operand) => 4x faster LDWEIGHTS (FWL) and a shorter
    post-h PE tail. Set WDT=bf16 to fall back.
  - xp scratch in DRAM as bf16 (halved traffic), scaled x16 to match.

Layouts as baseline: units on partitions; hist[l][d]: [128, (T+1)*8] bf16,
col = slot*8 + khalf*4 + b; fw writes slot t+1/reads t; bw writes t/reads
t+1; logits phase masks concat(fw1,bw1) then out_W matmul.
"""

import os
import numpy as np
import ml_dtypes

B, VOCAB, EMB, HID, OUT = 32, 256, 128, 256, 5
T_FULL = 1024
FORGET_BIAS = 1.0
NCORES = 8
BL = B // NCORES  # 4
CH = 64
POISON = 30.0

bf16 = ml_dtypes.bfloat16
_cache = {}


def _wdt():
    return os.environ.get("WDT", "bf16")


def _wscale():
    return 16.0 if _wdt() == "fp8" else 1.0


def _tile_lhsT(W, nk, nm):
    """[K=nk*128, M=nm*128] -> [128, nk*nm*128], col block (k*nm+m)."""
    return np.ascontiguousarray(
        W.reshape(nk, 128, nm, 128).transpose(1, 0, 2, 3).reshape(128, nk * nm * 128)
    )


def _patch_tile_drain(tile_mod, mybir):
    """Pinned walrus rejects >1 sync wait on a Drain; split extras onto NOPs."""
    if getattr(tile_mod, "_drain_patched", False):
        return

    def _drain_and_barrier(self, tick_clock, wait_clock):
        nc = self.nc
        drain_inst = nc.sync.drain()
        wait_clock.add_sem_waits(
            drain_inst.ins, tile_mod.ScopedClock({None: tick_clock.global_clock})
        )
        si = drain_inst.ins.sync_info
        if si is not None and len(si.on_wait) > 1:
            waits = list(si.on_wait)
            drain_inst.ins.sync_info = mybir.SyncInfo(
                on_wait=waits[:1], on_update=list(si.on_update)
            )
            for w in waits[1:]:
                nop = nc.sync.nop(nofuse=True, hint="drain_wait_split")
                nop.ins.sync_info = mybir.SyncInfo(on_wait=[w], on_update=[])
        nc.all_engine_barrier()
        assert self.sems is not None
        popped = nc._tile_sem_poison_stack.pop()
        assert popped is self._sem_poison
        nc.clear_and_free_semaphores(list(self.sems.allocated().values()))
        nc.all_engine_barrier()

    tile_mod.TileContext._drain_and_barrier = _drain_and_barrier
    tile_mod._drain_patched = True


def _patch_compiler_wait_split():
    """Pinned walrus accepts only 1 sync wait per instruction encoding slot
    it has available; rewrite the BIR before compiling so every instruction
    carries at most 1 wait, extras moved to preceding same-engine NoOps."""
    import json
    import concourse.bass_utils as bu
    import concourse.bass2jax as b2j

    if getattr(bu, "_wsplit_patched", False):
        return
    orig = bu.compile_bir_kernel

    def fix_block(bb, ctr):
        out = []
        for inst in bb.get("instructions", []):
            for blk in inst.get("blocks") or []:
                fix_block(blk, ctr)
            si = inst.get("sync_info")
            if si:
                ow = si.get("on_wait") or []
                if len(ow) > 1:
                    for w in ow[:-1]:
                        ctr[0] += 1
                        out.append(
                            {
                                "debug": inst.get("debug", 0),
                                "engine": inst["engine"],
                                "ins": [],
                                "name": f"wsplit-{ctr[0]}",
                                "opcode": "NoOp",
                                "outs": [],
                                "text_hint": "wsplit",
                                "sync_info": {"on_wait": [w], "on_update": []},
                            }
                        )
                    si["on_wait"] = [ow[-1]]
            out.append(inst)
        bb["instructions"] = out

    def wrapped(bir_json, tmpdir, neff_name="file.neff"):
        b = json.loads(bir_json)
        ctr = [0]
        for f in b.get("functions", []):
            for bb in f.get("blocks", []):
                fix_block(bb, ctr)
        return orig(json.dumps(b).encode(), tmpdir, neff_name)

    bu.compile_bir_kernel = wrapped
    b2j.compile_bir_kernel = wrapped

    if os.environ.get("LDW_OPT", "0") == "1":
        orig_run = bu.run_command

        def run_patched(argv, **kw):
            argv = [
                "--enable-ldw-opt=true" if a == "--enable-ldw-opt=false" else a
                for a in argv
            ]
            return orig_run(argv, **kw)

        bu.run_command = run_patched
    bu._wsplit_patched = True


def _build(T):
    import concourse.bass as bass
    import concourse.mybir as mybir
    import concourse.tile as tile

    _patch_tile_drain(tile, mybir)
    _patch_compiler_wait_split()
    f32 = mybir.dt.float32
    b16 = mybir.dt.bfloat16
    f16 = mybir.dt.float16
    w8 = mybir.dt.float8e3
    wdt = w8 if _wdt() == "fp8" else b16
    wnp_bytes = 1 if _wdt() == "fp8" else 2
    Sig = mybir.ActivationFunctionType.Sigmoid
    Tanh = mybir.ActivationFunctionType.Tanh
    ADD = mybir.AluOpType.add
    MUL = mybir.AluOpType.mult
    NT = T * BL
    WS = _wscale()

    nc = bass.Bass("TRN2", target_bir_lowering=False)

    x0T = nc.dram_tensor("x0T", [128, NT], b16, kind="ExternalInput")
    mask8 = nc.dram_tensor("mask8", [128, 8 * T], b16, kind="ExternalInput")
    whs, wxs, biases = {}, {}, {}
    for l in range(2):
        nk = 1 if l == 0 else 4
        for d in ("f", "b"):
            whs[(l, d)] = nc.dram_tensor(f"wh{l}{d}", [128, 16 * 128], wdt, kind="ExternalInput")
            wxs[(l, d)] = nc.dram_tensor(f"wx{l}{d}", [128, nk * 8 * 128], b16, kind="ExternalInput")
            biases[(l, d)] = nc.dram_tensor(f"bias{l}{d}", [128, 8], f32, kind="ExternalInput")
    outw = nc.dram_tensor("outw", [128, 20], b16, kind="ExternalInput")
    outb = nc.dram_tensor("outb", [128, 1], f32, kind="ExternalInput")
    identd = nc.dram_tensor("ident", [128, 128], f16, kind="ExternalInput")
    out = nc.dram_tensor("out", [5, NT], f32, kind="ExternalOutput")

    with tile.TileContext(nc) as tc:
        with tc.tile_pool(name="persist", bufs=1) as pp, \
             tc.tile_pool(name="xpbuf", bufs=2) as xpp, \
             tc.tile_pool(name="stage", bufs=3) as sp, \
             tc.tile_pool(name="small", bufs=2) as mp, \
             tc.tile_pool(name="zps", bufs=3, space="PSUM") as zp, \
             tc.tile_pool(name="pps", bufs=2, space="PSUM") as qp, \
             tc.tile_pool(name="xpd", bufs=1, space="DRAM") as dp:

            def load(name, dram, shape, dt):
                t = pp.tile(shape, dt, tag=name, name=name)
                nc.sync.dma_start(t[:], dram[:])
                return t

            x0T_s = load("x0T", x0T, [128, NT], b16)
            mask_s = load("mask8", mask8, [128, 8 * T], b16)
            wh_s = {k: load(f"wh{k[0]}{k[1]}", v, [128, 16 * 128], wdt) for k, v in whs.items()}
            wx_s = {k: load(f"wx{k[0]}{k[1]}", v, list(v.shape), b16) for k, v in wxs.items()}
            bias_s = {k: load(f"bias{k[0]}{k[1]}", v, [128, 8], f32) for k, v in biases.items()}
            outw_s = load("outw", outw, [128, 20], b16)
            outb_s = load("outb", outb, [128, 1], f32)

            # fp16 identity for the xp->PSUM inject matmul
            ident = load("ident", identd, [128, 128], f16)

            hist = {}
            for l in range(2):
                for d in ("f", "b"):
                    hist[(l, d)] = pp.tile([128, (T + 1) * 8], b16, tag=f"hist{l}{d}", name=f"hist{l}{d}")
            for l in range(2):
                nc.vector.memset(hist[(l, "f")][:, 0:8], 0.0)
                nc.vector.memset(hist[(l, "b")][:, T * 8 : T * 8 + 8], 0.0)

            # m-major layout: col = m*(T*4) + t*4 + b, so the XP phase's
            # [128,512] per-m tiles land as contiguous DRAM slices (the old
            # t-major layout produced 8-byte-strided writes: ~16K descriptors
            # = ~30us per tile, which starved the scans).
            xp_dram = {
                (l, d): dp.tile([128, 8 * T * 4], f16, tag=f"xp{l}{d}", name=f"xp{l}{d}")
                for l in range(2)
                for d in ("f", "b")
            }

            def hist_rhs(l, d, half, c0):
                """[128, 128t, 4b] output slice of a history for XP1/logits.
                fw output for position t is slot t+1; bw output is slot t."""
                r = hist[(l, d)].rearrange("p (s q) -> p s q", q=8)
                s0 = c0 + 1 if d == "f" else c0
                return r[:, s0 : s0 + 128, half * 4 : half * 4 + 4]

            # poison per m-slot (gate order i,i,j,j -> wait: TF order i,j,f,o
            # => m-slots 0,1=i  2,3=j  4,5=f  6,7=o). Only i and f need
            # poisoning: i -> -POISON (u term ~ 0), f -> +POISON (c carries).
            PVAL = {0: -POISON * WS, 1: -POISON * WS, 4: POISON * WS, 5: POISON * WS}

            def xp_phase(l, d):
                nk = 1 if l == 0 else 4
                wx_t = wx_s[(l, d)]
                xp_r = xp_dram[(l, d)].rearrange("p (m s) -> p m s", m=8)
                mask_r = mask_s.rearrange("p (s q) -> p s q", q=8)
                for c0 in range(0, T, 128):
                    for m in range(8):
                        ps = qp.tile([128, 512], f32, tag="proj_ps")
                        for k in range(nk):
                            if l == 0:
                                rhs = x0T_s[:, c0 * 4 : c0 * 4 + 512]
                            else:
                                rhs = hist_rhs(0, "f" if k < 2 else "b", k % 2, c0)
                            nc.tensor.matmul(
                                ps[:],
                                wx_t[:, (k * 8 + m) * 128 : (k * 8 + m + 1) * 128],
                                rhs,
                                start=(k == 0),
                                stop=(k == nk - 1),
                            )
                        st = sp.tile([128, 512], f16, tag="xp_st")
                        if d == "b" and m in PVAL:
                            p = PVAL[m]
                            # xp' = (ps + bias - p) * mask + p
                            s1 = sp.tile([128, 512], f32, tag="xp_s1")
                            nc.vector.tensor_scalar_add(s1[:], ps[:], biasP_s[(l, d)][:, m : m + 1])
                            mview = mask_r[:, c0 : c0 + 128, 0:4]
                            nc.vector.tensor_tensor(
                                st.rearrange("p (s q) -> p s q", q=4)[:],
                                s1.rearrange("p (s q) -> p s q", q=4)[:],
                                mview, MUL,
                            )
                            nc.vector.tensor_scalar_add(st[:], st[:], float(p))
                        else:
                            nc.vector.tensor_scalar_add(st[:], ps[:], bias_s[(l, d)][:, m : m + 1])
                        nc.sync.dma_start(xp_r[:, m, c0 * 4 : c0 * 4 + 512], st[:])

            # bias - poison tiles for the poisoned m-slots (per l, bw only)
            biasP_s = {}
            for l in range(2):
                biasP_s[(l, "b")] = pp.tile([128, 8], f32, tag=f"biasP{l}", name=f"biasP{l}")
                bb = biasP_s[(l, "b")]
                nc.vector.tensor_copy(bb[:], bias_s[(l, "b")][:])
                for m, p in PVAL.items():
                    nc.vector.tensor_scalar_add(bb[:, m : m + 1], bb[:, m : m + 1], float(-p))

            def step(l, d, t, xp_tile, i):
                """One LSTM step: z = wh.T@h + xp (PSUM), s = sigmoid(z/WS),
                c = s_f*c + s_i*(2*s_j-1), h = s_o*tanh(c)."""
                h = hist[(l, d)]
                if d == "f":
                    r_off, w_off = t * 8, (t + 1) * 8
                else:
                    r_off, w_off = (t + 1) * 8, t * 8
                z = zp.tile([128, 32], f32, tag=f"z{d}")
                xr = xp_tile.rearrange("p (m s q) -> p m s q", m=8, q=4)
                nc.tensor.matmul(
                    z[:], ident[:], xr[:, :, i, :],
                    start=True, stop=False, skip_group_check=True,
                )
                wh_t = wh_s[(l, d)]
                for m in range(8):
                    for k in range(2):
                        nc.tensor.matmul(
                            z[:, 4 * m : 4 * m + 4],
                            wh_t[:, (k * 8 + m) * 128 : (k * 8 + m + 1) * 128],
                            h[:, r_off + 4 * k : r_off + 4 * k + 4],
                            start=False,
                            stop=(m == 7 and k == 1),
                            skip_group_check=True,
                        )
                s = sp.tile([128, 32], f32, tag=f"s{d}")
                nc.scalar.activation(s[:], z[:], Sig, scale=1.0 / WS)
                c = cs[(l, d)]
                tt = mp.tile([128, 8], f32, tag=f"tt{d}")
                # tt = (s_j - 0.5) * s_i
                nc.vector.scalar_tensor_tensor(tt[:], s[:, 8:16], -0.5, s[:, 0:8], ADD, MUL)
                c1 = mp.tile([128, 8], f32, tag=f"c1{d}")
                nc.vector.tensor_tensor(c1[:], s[:, 16:24], c[:], MUL)
                # c = 2*tt + c1
                nc.vector.scalar_tensor_tensor(c[:], tt[:], 2.0, c1[:], MUL, ADD)
                th = mp.tile([128, 8], f32, tag=f"th{d}")
                nc.scalar.activation(th[:], c[:], Tanh)
                nc.vector.tensor_tensor(h[:, w_off : w_off + 8], s[:, 24:32], th[:], MUL)

            cs = {}
            for l in range(2):
                for d in ("f", "b"):
                    cs[(l, d)] = pp.tile([128, 8], f32, tag=f"c{l}{d}", name=f"c{l}{d}")

            for l in range(2):
                xp_phase(l, "f")
                xp_phase(l, "b")
                nc.vector.memset(cs[(l, "f")][:], 0.0)
                nc.vector.memset(cs[(l, "b")][:], 0.0)
                for c0 in range(0, T, CH):
                    # chunk tiles are m-major: [128, m(8), CH*4]
                    xf = xpp.tile([128, 8 * CH * 4], f16, tag="xf")
                    src_f = xp_dram[(l, "f")].rearrange("p (m s) -> p m s", m=8)
                    nc.sync.dma_start(
                        xf.rearrange("p (m s) -> p m s", m=8)[:],
                        src_f[:, :, c0 * 4 : (c0 + CH) * 4],
                    )
                    xb = xpp.tile([128, 8 * CH * 4], f16, tag="xb")
                    bw_lo = T - c0 - CH
                    src_b = xp_dram[(l, "b")].rearrange("p (m s) -> p m s", m=8)
                    nc.sync.dma_start(
                        xb.rearrange("p (m s) -> p m s", m=8)[:],
                        src_b[:, :, bw_lo * 4 : (bw_lo + CH) * 4],
                    )
                    for i in range(CH):
                        step(l, "f", c0 + i, xf, i)
                        step(l, "b", T - 1 - (c0 + i), xb, CH - 1 - i)

            # ---- logits ----
            mask_r = mask_s.rearrange("p (s q) -> p s q", q=8)
            for c0 in range(0, T, 128):
                ps = qp.tile([128, 512], f32, tag="proj_ps")
                for k in range(4):
                    rhs = hist_rhs(1, "f" if k < 2 else "b", k % 2, c0)
                    nc.tensor.matmul(
                        ps[:5, :],
                        outw_s[:, k * 5 : k * 5 + 5],
                        rhs,
                        start=(k == 0),
                        stop=(k == 3),
                    )
                lg = sp.tile([5, 512], f32, tag="lg")
                nc.vector.tensor_mul(lg[:], ps[:5, :], mask_r[:5, c0 : c0 + 128, 0:4])
                nc.vector.tensor_scalar_add(lg[:], lg[:], outb_s[:5, 0:1])
                nc.sync.dma_start(out[:, c0 * 4 : c0 * 4 + 512], lg[:])

    return nc


last_results = None


def kernel(**inputs):
    global last_results
    T = int(os.environ.get("KERNEL_T", T_FULL))
    from concourse.bass_utils import run_bass_kernel_spmd

    tokens = np.asarray(inputs["tokens"])[:, :T]
    lengths = np.clip(np.asarray(inputs["lengths"]), 0, T)
    emb = np.asarray(inputs["emb"], dtype=np.float32)
    WS = _wscale()
    w8np = ml_dtypes.float8_e3m4

    if T not in _cache:
        _cache[T] = _build(T)
    nc = _cache[T]

    # ---- host-side retiling (shared across cores) ----
    shared = {}
    for l in range(2):
        D = EMB if l == 0 else 2 * HID
        nk = D // 128
        for d, pre in (("f", "fw"), ("b", "bw")):
            W = np.asarray(inputs[f"{pre}_W{l}"], dtype=np.float32).copy()
            bias = np.asarray(inputs[f"{pre}_b{l}"], dtype=np.float32).copy()
            bias[2 * HID : 3 * HID] += FORGET_BIAS
            # double the j-gate columns: sigmoid(2*z_j) = (tanh(z_j)+1)/2
            W[:, HID : 2 * HID] *= 2.0
            bias[HID : 2 * HID] *= 2.0
            wh = _tile_lhsT(W[D:] * WS, 2, 8)
            if _wdt() == "fp8":
                shared[f"wh{l}{d}"] = wh.astype(w8np)
            else:
                shared[f"wh{l}{d}"] = wh.astype(bf16)
            shared[f"wx{l}{d}"] = _tile_lhsT(W[:D] * WS, nk, 8).astype(bf16)
            shared[f"bias{l}{d}"] = np.ascontiguousarray(
                (bias * WS).reshape(8, 128).T
            ).astype(np.float32)
    shared["outw"] = np.ascontiguousarray(
        np.asarray(inputs["out_W"], dtype=np.float32)
        .reshape(4, 128, 5)
        .transpose(1, 0, 2)
        .reshape(128, 20)
    ).astype(bf16)
    ob = np.zeros((128, 1), np.float32)
    ob[:5, 0] = np.asarray(inputs["out_b"], dtype=np.float32)
    shared["outb"] = ob
    shared["ident"] = np.eye(128, dtype=np.float16)

    in_maps = []
    for ci in range(NCORES):
        bs = slice(ci * BL, (ci + 1) * BL)
        x0 = emb[tokens[bs]]  # [BL, T, 128]
        x0T = np.ascontiguousarray(x0.transpose(2, 1, 0).reshape(128, T * BL)).astype(bf16)
        mvec = (np.arange(T)[:, None] < lengths[bs][None, :]).astype(bf16)  # [T, BL]
        m8 = np.repeat(mvec[:, None, :], 2, axis=1).reshape(1, T * 8)
        mask8 = np.ascontiguousarray(np.broadcast_to(m8, (128, T * 8)))
        im = dict(shared)
        im["x0T"] = x0T
        im["mask8"] = mask8
        in_maps.append(im)

    res = run_bass_kernel_spmd(nc, in_maps, core_ids=list(range(NCORES)))
    last_results = res
    outs = []
    for ci in range(NCORES):
        o = res.results[ci]["out"]  # [5, T*BL]
        outs.append(o.reshape(5, T, BL).transpose(2, 1, 0))  # [BL, T, 5]
    return np.concatenate(outs, axis=0).astype(np.float32)


# revision 27
# speedup vs baseline: 3.1189x; 1.0152x over previous
"""Trainium2 Bass kernel for a 2-layer bidirectional LSTM char model (B=32,
T=1024, EMB=128, HID=256, OUT=5).

kernel(**inputs) takes the FULL unsharded inputs, returns FULL [B,T,5] f32
logits. Data-parallel over batch on 8 NeuronCores (4 examples/core); each
core runs all four scans (2 layers x 2 dirs), fw/bw interleaved per step so
engines pipeline across the two independent chains.

v2 redesign — minimize the per-step critical path (wall = 2048 sequential
cell steps x step latency):
  - xp (input projection + bias) is injected into the z PSUM tile via an
    identity matmul at the head of the accumulation group; it has no h
    dependency so it runs during the h-wait, and the PSUM-resident z means
    no zs-add on the critical path.
  - ONE sigmoid activation covers all four gates: the j-gate's weights,
    bias and xp are pre-doubled host-side so sigmoid(2*z_j) gives
    tanh(z_j) = 2*sigmoid(2*z_j) - 1, recovered inside the DVE chain.
  - c-update in 3 DVE ops using scalar_tensor_tensor:
        t  = (s_j - 0.5) * s_i          # = u/2
        c1 = s_f * c_prev
        c  = 2*t + c1
  - bw sequence masking via xp POISONING instead of per-step mask muls:
    for masked positions (t >= len), xp is overwritten so z_i = -30 and
    z_f = +30, making c (and hence h) carry through unchanged (state stays
    exactly 0 through the masked prefix of the bw scan). No mask ops on the
    recurrent chain at all.
  - recurrent weights stored fp8 e3m4 (x16 scaled; descaled for free via
    the sigmoid's scale operand) => 4x faster LDWEIGHTS (FWL) and a shorter
    post-h PE tail. Set WDT=bf16 to fall back.
  - xp scratch in DRAM as bf16 (halved traffic), scaled x16 to match.

Layouts as baseline: units on partitions; hist[l][d]: [128, (T+1)*8] bf16,
col = slot*8 + khalf*4 + b; fw writes slot t+1/reads t; bw writes t/reads
t+1; logits phase masks concat(fw1,bw1) then out_W matmul.
"""

import os
import numpy as np
import ml_dtypes

B, VOCAB, EMB, HID, OUT = 32, 256, 128, 256, 5
T_FULL = 1024
FORGET_BIAS = 1.0
NCORES = 8
BL = B // NCORES  # 4
CH = 64
POISON = 30.0

bf16 = ml_dtypes.bfloat16
_cache = {}


def _wdt():
    return os.environ.get("WDT", "bf16")


def _wscale():
    return 16.0 if _wdt() == "fp8" else 1.0


def _tile_lhsT(W, nk, nm):
    """[K=nk*128, M=nm*128] -> [128, nk*nm*128], col block (k*nm+m)."""
    return np.ascontiguousarray(
        W.reshape(nk, 128, nm, 128).transpose(1, 0, 2, 3).reshape(128, nk * nm * 128)
    )


def _patch_tile_drain(tile_mod, mybir):
    """Pinned walrus rejects >1 sync wait on a Drain; split extras onto NOPs."""
    if getattr(tile_mod, "_drain_patched", False):
        return

    def _drain_and_barrier(self, tick_clock, wait_clock):
        nc = self.nc
        drain_inst = nc.sync.drain()
        wait_clock.add_sem_waits(
            drain_inst.ins, tile_mod.ScopedClock({None: tick_clock.global_clock})
        )
        si = drain_inst.ins.sync_info
        if si is not None and len(si.on_wait) > 1:
            waits = list(si.on_wait)
            drain_inst.ins.sync_info = mybir.SyncInfo(
                on_wait=waits[:1], on_update=list(si.on_update)
            )
            for w in waits[1:]:
                nop = nc.sync.nop(nofuse=True, hint="drain_wait_split")
                nop.ins.sync_info = mybir.SyncInfo(on_wait=[w], on_update=[])
        nc.all_engine_barrier()
        assert self.sems is not None
        popped = nc._tile_sem_poison_stack.pop()
        assert popped is self._sem_poison
        nc.clear_and_free_semaphores(list(self.sems.allocated().values()))
        nc.all_engine_barrier()

    tile_mod.TileContext._drain_and_barrier = _drain_and_barrier
    tile_mod._drain_patched = True


def _patch_compiler_wait_split():
    """Pinned walrus accepts only 1 sync wait per instruction encoding slot
    it has available; rewrite the BIR before compiling so every instruction
    carries at most 1 wait, extras moved to preceding same-engine NoOps."""
    import json
    import concourse.bass_utils as bu
    import concourse.bass2jax as b2j

    if getattr(bu, "_wsplit_patched", False):
        return
    orig = bu.compile_bir_kernel

    def fix_block(bb, ctr):
        out = []
        for inst in bb.get("instructions", []):
            for blk in inst.get("blocks") or []:
                fix_block(blk, ctr)
            si = inst.get("sync_info")
            if si:
                ow = si.get("on_wait") or []
                if len(ow) > 1:
                    for w in ow[:-1]:
                        ctr[0] += 1
                        out.append(
                            {
                                "debug": inst.get("debug", 0),
                                "engine": inst["engine"],
                                "ins": [],
                                "name": f"wsplit-{ctr[0]}",
                                "opcode": "NoOp",
                                "outs": [],
                                "text_hint": "wsplit",
                                "sync_info": {"on_wait": [w], "on_update": []},
                            }
                        )
                    si["on_wait"] = [ow[-1]]
            out.append(inst)
        bb["instructions"] = out

    def wrapped(bir_json, tmpdir, neff_name="file.neff"):
        b = json.loads(bir_json)
        ctr = [0]
        for f in b.get("functions", []):
            for bb in f.get("blocks", []):
                fix_block(bb, ctr)
        return orig(json.dumps(b).encode(), tmpdir, neff_name)

    bu.compile_bir_kernel = wrapped
    b2j.compile_bir_kernel = wrapped

    if os.environ.get("LDW_OPT", "0") == "1":
        orig_run = bu.run_command

        def run_patched(argv, **kw):
            argv = [
                "--enable-ldw-opt=true" if a == "--enable-ldw-opt=false" else a
                for a in argv
            ]
            return orig_run(argv, **kw)

        bu.run_command = run_patched
    bu._wsplit_patched = True


def _build(T):
    import concourse.bass as bass
    import concourse.mybir as mybir
    import concourse.tile as tile

    _patch_tile_drain(tile, mybir)
    _patch_compiler_wait_split()
    f32 = mybir.dt.float32
    b16 = mybir.dt.bfloat16
    f16 = mybir.dt.float16
    w8 = mybir.dt.float8e3
    wdt = w8 if _wdt() == "fp8" else b16
    wnp_bytes = 1 if _wdt() == "fp8" else 2
    Sig = mybir.ActivationFunctionType.Sigmoid
    Tanh = mybir.ActivationFunctionType.Tanh
    ADD = mybir.AluOpType.add
    MUL = mybir.AluOpType.mult
    NT = T * BL
    WS = _wscale()

    nc = bass.Bass("TRN2", target_bir_lowering=False)

    x0T = nc.dram_tensor("x0T", [128, NT], b16, kind="ExternalInput")
    mask8 = nc.dram_tensor("mask8", [128, 8 * T], b16, kind="ExternalInput")
    whs, wxs, biases = {}, {}, {}
    for l in range(2):
        nk = 1 if l == 0 else 4
        for d in ("f", "b"):
            whs[(l, d)] = nc.dram_tensor(f"wh{l}{d}", [128, 16 * 128], wdt, kind="ExternalInput")
            wxs[(l, d)] = nc.dram_tensor(f"wx{l}{d}", [128, nk * 8 * 128], b16, kind="ExternalInput")
            biases[(l, d)] = nc.dram_tensor(f"bias{l}{d}", [128, 8], f32, kind="ExternalInput")
    outw = nc.dram_tensor("outw", [128, 20], b16, kind="ExternalInput")
    outb = nc.dram_tensor("outb", [128, 1], f32, kind="ExternalInput")
    identd = nc.dram_tensor("ident", [128, 128], f16, kind="ExternalInput")
    out = nc.dram_tensor("out", [5, NT], f32, kind="ExternalOutput")

    with tile.TileContext(nc) as tc:
        with tc.tile_pool(name="persist", bufs=1) as pp, \
             tc.tile_pool(name="xpbuf", bufs=2) as xpp, \
             tc.tile_pool(name="stage", bufs=3) as sp, \
             tc.tile_pool(name="small", bufs=2) as mp, \
             tc.tile_pool(name="zps", bufs=3, space="PSUM") as zp, \
             tc.tile_pool(name="pps", bufs=2, space="PSUM") as qp, \
             tc.tile_pool(name="xpd", bufs=1, space="DRAM") as dp:

            def load(name, dram, shape, dt):
                t = pp.tile(shape, dt, tag=name, name=name)
                nc.sync.dma_start(t[:], dram[:])
                return t

            x0T_s = load("x0T", x0T, [128, NT], b16)
            mask_s = load("mask8", mask8, [128, 8 * T], b16)
            wh_s = {k: load(f"wh{k[0]}{k[1]}", v, [128, 16 * 128], wdt) for k, v in whs.items()}
            wx_s = {k: load(f"wx{k[0]}{k[1]}", v, list(v.shape), b16) for k, v in wxs.items()}
            bias_s = {k: load(f"bias{k[0]}{k[1]}", v, [128, 8], f32) for k, v in biases.items()}
            outw_s = load("outw", outw, [128, 20], b16)
            outb_s = load("outb", outb, [128, 1], f32)

            # fp16 identity for the xp->PSUM inject matmul
            ident = load("ident", identd, [128, 128], f16)

            hist = {}
            for l in range(2):
                for d in ("f", "b"):
                    hist[(l, d)] = pp.tile([128, (T + 1) * 8], b16, tag=f"hist{l}{d}", name=f"hist{l}{d}")
            for l in range(2):
                nc.vector.memset(hist[(l, "f")][:, 0:8], 0.0)
                nc.vector.memset(hist[(l, "b")][:, T * 8 : T * 8 + 8], 0.0)

            # m-major layout: col = m*(Tseg*4) + t_local*4 + b, so the XP
            # phase's [128,512] per-m tiles land as contiguous DRAM slices
            # (a t-major layout produces 8-byte-strided writes: ~16K
            # descriptors = ~30us per tile, which starved the scans).
            # Segmented 4x per (l,d): finer DRAM write->read dependencies
            # let scan chunks chase the XP phase instead of waiting for the
            # whole projection to finish.
            TSEG = max(T // 4, 128)
            NSEG = T // TSEG
            xp_dram = {
                (l, d): [
                    dp.tile([128, 8 * TSEG * 4], f16, tag=f"xp{l}{d}{g}", name=f"xp{l}{d}{g}")
                    for g in range(NSEG)
                ]
                for l in range(2)
                for d in ("f", "b")
            }

            def hist_rhs(l, d, half, c0):
                """[128, 128t, 4b] output slice of a history for XP1/logits.
                fw output for position t is slot t+1; bw output is slot t."""
                r = hist[(l, d)].rearrange("p (s q) -> p s q", q=8)
                s0 = c0 + 1 if d == "f" else c0
                return r[:, s0 : s0 + 128, half * 4 : half * 4 + 4]

            # poison per m-slot (gate order i,i,j,j -> wait: TF order i,j,f,o
            # => m-slots 0,1=i  2,3=j  4,5=f  6,7=o). Only i and f need
            # poisoning: i -> -POISON (u term ~ 0), f -> +POISON (c carries).
            PVAL = {0: -POISON * WS, 1: -POISON * WS, 4: POISON * WS, 5: POISON * WS}

            def xp_phase(l, d):
                nk = 1 if l == 0 else 4
                wx_t = wx_s[(l, d)]
                mask_r = mask_s.rearrange("p (s q) -> p s q", q=8)
                # bw scans consume high-t chunks first: emit those first so
                # the bw chain starts as soon as its tail segments exist
                c0s = list(range(0, T, 128))
                if d == "b":
                    c0s.reverse()
                for c0 in c0s:
                    xp_r = xp_dram[(l, d)][c0 // TSEG].rearrange("p (m s) -> p m s", m=8)
                    for m in range(8):
                        ps = qp.tile([128, 512], f32, tag="proj_ps")
                        for k in range(nk):
                            if l == 0:
                                rhs = x0T_s[:, c0 * 4 : c0 * 4 + 512]
                            else:
                                rhs = hist_rhs(0, "f" if k < 2 else "b", k % 2, c0)
                            nc.tensor.matmul(
                                ps[:],
                                wx_t[:, (k * 8 + m) * 128 : (k * 8 + m + 1) * 128],
                                rhs,
                                start=(k == 0),
                                stop=(k == nk - 1),
                            )
                        st = sp.tile([128, 512], f16, tag="xp_st")
                        if d == "b" and m in PVAL:
                            p = PVAL[m]
                            # xp' = (ps + bias - p) * mask + p
                            s1 = sp.tile([128, 512], f32, tag="xp_s1")
                            nc.vector.tensor_scalar_add(s1[:], ps[:], biasP_s[(l, d)][:, m : m + 1])
                            mview = mask_r[:, c0 : c0 + 128, 0:4]
                            nc.vector.tensor_tensor(
                                st.rearrange("p (s q) -> p s q", q=4)[:],
                                s1.rearrange("p (s q) -> p s q", q=4)[:],
                                mview, MUL,
                            )
                            nc.vector.tensor_scalar_add(st[:], st[:], float(p))
                        else:
                            nc.vector.tensor_scalar_add(st[:], ps[:], bias_s[(l, d)][:, m : m + 1])
                        lc = (c0 % TSEG) * 4
                        nc.sync.dma_start(xp_r[:, m, lc : lc + 512], st[:])

            # bias - poison tiles for the poisoned m-slots (per l, bw only)
            biasP_s = {}
            for l in range(2):
                biasP_s[(l, "b")] = pp.tile([128, 8], f32, tag=f"biasP{l}", name=f"biasP{l}")
                bb = biasP_s[(l, "b")]
                nc.vector.tensor_copy(bb[:], bias_s[(l, "b")][:])
                for m, p in PVAL.items():
                    nc.vector.tensor_scalar_add(bb[:, m : m + 1], bb[:, m : m + 1], float(-p))

            def step(l, d, t, xp_tile, i):
                """One LSTM step: z = wh.T@h + xp (PSUM), s = sigmoid(z/WS),
                c = s_f*c + s_i*(2*s_j-1), h = s_o*tanh(c)."""
                h = hist[(l, d)]
                if d == "f":
                    r_off, w_off = t * 8, (t + 1) * 8
                else:
                    r_off, w_off = (t + 1) * 8, t * 8
                z = zp.tile([128, 32], f32, tag=f"z{d}")
                xr = xp_tile.rearrange("p (m s q) -> p m s q", m=8, q=4)
                nc.tensor.matmul(
                    z[:], ident[:], xr[:, :, i, :],
                    start=True, stop=False, skip_group_check=True,
                )
                wh_t = wh_s[(l, d)]
                for k in range(2):
                    for m in range(8):
                        nc.tensor.matmul(
                            z[:, 4 * m : 4 * m + 4],
                            wh_t[:, (k * 8 + m) * 128 : (k * 8 + m + 1) * 128],
                            h[:, r_off + 4 * k : r_off + 4 * k + 4],
                            start=False,
                            stop=(m == 7 and k == 1),
                            skip_group_check=True,
                        )
                s = sp.tile([128, 32], f32, tag=f"s{d}")
                nc.scalar.activation(s[:], z[:], Sig, scale=1.0 / WS)
                c = cs[(l, d)]
                tt = mp.tile([128, 8], f32, tag=f"tt{d}")
                # tt = (s_j - 0.5) * s_i
                nc.vector.scalar_tensor_tensor(tt[:], s[:, 8:16], -0.5, s[:, 0:8], ADD, MUL)
                c1 = mp.tile([128, 8], f32, tag=f"c1{d}")
                nc.vector.tensor_tensor(c1[:], s[:, 16:24], c[:], MUL)
                # c = 2*tt + c1
                nc.vector.scalar_tensor_tensor(c[:], tt[:], 2.0, c1[:], MUL, ADD)
                th = mp.tile([128, 8], f32, tag=f"th{d}")
                nc.scalar.activation(th[:], c[:], Tanh)
                nc.vector.tensor_tensor(h[:, w_off : w_off + 8], s[:, 24:32], th[:], MUL)

            cs = {}
            for l in range(2):
                for d in ("f", "b"):
                    cs[(l, d)] = pp.tile([128, 8], f32, tag=f"c{l}{d}", name=f"c{l}{d}")

            for l in range(2):
                xp_phase(l, "f")
                xp_phase(l, "b")
                nc.vector.memset(cs[(l, "f")][:], 0.0)
                nc.vector.memset(cs[(l, "b")][:], 0.0)
                for c0 in range(0, T, CH):
                    # chunk tiles are m-major: [128, m(8), CH*4]
                    xf = xpp.tile([128, 8 * CH * 4], f16, tag="xf")
                    src_f = xp_dram[(l, "f")][c0 // TSEG].rearrange("p (m s) -> p m s", m=8)
                    lf = (c0 % TSEG) * 4
                    nc.sync.dma_start(
                        xf.rearrange("p (m s) -> p m s", m=8)[:],
                        src_f[:, :, lf : lf + CH * 4],
                    )
                    xb = xpp.tile([128, 8 * CH * 4], f16, tag="xb")
                    bw_lo = T - c0 - CH
                    src_b = xp_dram[(l, "b")][bw_lo // TSEG].rearrange("p (m s) -> p m s", m=8)
                    lb = (bw_lo % TSEG) * 4
                    nc.sync.dma_start(
                        xb.rearrange("p (m s) -> p m s", m=8)[:],
                        src_b[:, :, lb : lb + CH * 4],
                    )
                    for i in range(CH):
                        step(l, "f", c0 + i, xf, i)
                        step(l, "b", T - 1 - (c0 + i), xb, CH - 1 - i)

            # ---- logits ----
            mask_r = mask_s.rearrange("p (s q) -> p s q", q=8)
            for c0 in range(0, T, 128):
                ps = qp.tile([128, 512], f32, tag="proj_ps")
                for k in range(4):
                    rhs = hist_rhs(1, "f" if k < 2 else "b", k % 2, c0)
                    nc.tensor.matmul(
                        ps[:5, :],
                        outw_s[:, k * 5 : k * 5 + 5],
                        rhs,
                        start=(k == 0),
                        stop=(k == 3),
                    )
                lg = sp.tile([5, 512], f32, tag="lg")
                nc.vector.tensor_mul(lg[:], ps[:5, :], mask_r[:5, c0 : c0 + 128, 0:4])
                nc.vector.tensor_scalar_add(lg[:], lg[:], outb_s[:5, 0:1])
                nc.sync.dma_start(out[:, c0 * 4 : c0 * 4 + 512], lg[:])

    return nc


last_results = None


def kernel(**inputs):
    global last_results
    T = int(os.environ.get("KERNEL_T", T_FULL))
    from concourse.bass_utils import run_bass_kernel_spmd

    tokens = np.asarray(inputs["tokens"])[:, :T]
    lengths = np.clip(np.asarray(inputs["lengths"]), 0, T)
    emb = np.asarray(inputs["emb"], dtype=np.float32)
    WS = _wscale()
    w8np = ml_dtypes.float8_e3m4

    if T not in _cache:
        _cache[T] = _build(T)
    nc = _cache[T]

    # ---- host-side retiling (shared across cores) ----
    shared = {}
    for l in range(2):
        D = EMB if l == 0 else 2 * HID
        nk = D // 128
        for d, pre in (("f", "fw"), ("b", "bw")):
            W = np.asarray(inputs[f"{pre}_W{l}"], dtype=np.float32).copy()
            bias = np.asarray(inputs[f"{pre}_b{l}"], dtype=np.float32).copy()
            bias[2 * HID : 3 * HID] += FORGET_BIAS
            # double the j-gate columns: sigmoid(2*z_j) = (tanh(z_j)+1)/2
            W[:, HID : 2 * HID] *= 2.0
            bias[HID : 2 * HID] *= 2.0
            wh = _tile_lhsT(W[D:] * WS, 2, 8)
            if _wdt() == "fp8":
                shared[f"wh{l}{d}"] = wh.astype(w8np)
            else:
                shared[f"wh{l}{d}"] = wh.astype(bf16)
            shared[f"wx{l}{d}"] = _tile_lhsT(W[:D] * WS, nk, 8).astype(bf16)
            shared[f"bias{l}{d}"] = np.ascontiguousarray(
                (bias * WS).reshape(8, 128).T
            ).astype(np.float32)
    shared["outw"] = np.ascontiguousarray(
        np.asarray(inputs["out_W"], dtype=np.float32)
        .reshape(4, 128, 5)
        .transpose(1, 0, 2)
        .reshape(128, 20)
    ).astype(bf16)
    ob = np.zeros((128, 1), np.float32)
    ob[:5, 0] = np.asarray(inputs["out_b"], dtype=np.float32)
    shared["outb"] = ob
    shared["ident"] = np.eye(128, dtype=np.float16)

    in_maps = []
    for ci in range(NCORES):
        bs = slice(ci * BL, (ci + 1) * BL)
        x0 = emb[tokens[bs]]  # [BL, T, 128]
        x0T = np.ascontiguousarray(x0.transpose(2, 1, 0).reshape(128, T * BL)).astype(bf16)
        mvec = (np.arange(T)[:, None] < lengths[bs][None, :]).astype(bf16)  # [T, BL]
        m8 = np.repeat(mvec[:, None, :], 2, axis=1).reshape(1, T * 8)
        mask8 = np.ascontiguousarray(np.broadcast_to(m8, (128, T * 8)))
        im = dict(shared)
        im["x0T"] = x0T
        im["mask8"] = mask8
        in_maps.append(im)

    res = run_bass_kernel_spmd(nc, in_maps, core_ids=list(range(NCORES)))
    last_results = res
    outs = []
    for ci in range(NCORES):
        o = res.results[ci]["out"]  # [5, T*BL]
        outs.append(o.reshape(5, T, BL).transpose(2, 1, 0))  # [BL, T, 5]
    return np.concatenate(outs, axis=0).astype(np.float32)


# revision 30
# speedup vs baseline: 3.1611x; 1.0135x over previous
"""Trainium2 Bass kernel for a 2-layer bidirectional LSTM char model (B=32,
T=1024, EMB=128, HID=256, OUT=5).

kernel(**inputs) takes the FULL unsharded inputs, returns FULL [B,T,5] f32
logits. Data-parallel over batch on 8 NeuronCores (4 examples/core); each
core runs all four scans (2 layers x 2 dirs), fw/bw interleaved per step so
engines pipeline across the two independent chains.

v2 redesign — minimize the per-step critical path (wall = 2048 sequential
cell steps x step latency):
  - xp (input projection + bias) is injected into the z PSUM tile via an
    identity matmul at the head of the accumulation group; it has no h
    dependency so it runs during the h-wait, and the PSUM-resident z means
    no zs-add on the critical path.
  - ONE sigmoid activation covers all four gates: the j-gate's weights,
    bias and xp are pre-doubled host-side so sigmoid(2*z_j) gives
    tanh(z_j) = 2*sigmoid(2*z_j) - 1, recovered inside the DVE chain.
  - c-update in 3 DVE ops using scalar_tensor_tensor:
        t  = (s_j - 0.5) * s_i          # = u/2
        c1 = s_f * c_prev
        c  = 2*t + c1
  - bw sequence masking via xp POISONING instead of per-step mask muls:
    for masked positions (t >= len), xp is overwritten so z_i = -30 and
    z_f = +30, making c (and hence h) carry through unchanged (state stays
    exactly 0 through the masked prefix of the bw scan). No mask ops on the
    recurrent chain at all.
  - recurrent weights stored fp8 e3m4 (x16 scaled; descaled for free via
    the sigmoid's scale operand) => 4x faster LDWEIGHTS (FWL) and a shorter
    post-h PE tail. Set WDT=bf16 to fall back.
  - xp scratch in DRAM as bf16 (halved traffic), scaled x16 to match.

Layouts as baseline: units on partitions; hist[l][d]: [128, (T+1)*8] bf16,
col = slot*8 + khalf*4 + b; fw writes slot t+1/reads t; bw writes t/reads
t+1; logits phase masks concat(fw1,bw1) then out_W matmul.
"""

import os
import numpy as np
import ml_dtypes

B, VOCAB, EMB, HID, OUT = 32, 256, 128, 256, 5
T_FULL = 1024
FORGET_BIAS = 1.0
NCORES = 8
BL = B // NCORES  # 4
CH = 64
POISON = 30.0

bf16 = ml_dtypes.bfloat16
_cache = {}


def _wdt():
    return os.environ.get("WDT", "bf16")


def _wscale():
    return 16.0 if _wdt() == "fp8" else 1.0


def _tile_lhsT(W, nk, nm):
    """[K=nk*128, M=nm*128] -> [128, nk*nm*128], col block (k*nm+m)."""
    return np.ascontiguousarray(
        W.reshape(nk, 128, nm, 128).transpose(1, 0, 2, 3).reshape(128, nk * nm * 128)
    )


def _patch_tile_drain(tile_mod, mybir):
    """Pinned walrus rejects >1 sync wait on a Drain; split extras onto NOPs."""
    if getattr(tile_mod, "_drain_patched", False):
        return

    def _drain_and_barrier(self, tick_clock, wait_clock):
        nc = self.nc
        drain_inst = nc.sync.drain()
        wait_clock.add_sem_waits(
            drain_inst.ins, tile_mod.ScopedClock({None: tick_clock.global_clock})
        )
        si = drain_inst.ins.sync_info
        if si is not None and len(si.on_wait) > 1:
            waits = list(si.on_wait)
            drain_inst.ins.sync_info = mybir.SyncInfo(
                on_wait=waits[:1], on_update=list(si.on_update)
            )
            for w in waits[1:]:
                nop = nc.sync.nop(nofuse=True, hint="drain_wait_split")
                nop.ins.sync_info = mybir.SyncInfo(on_wait=[w], on_update=[])
        nc.all_engine_barrier()
        assert self.sems is not None
        popped = nc._tile_sem_poison_stack.pop()
        assert popped is self._sem_poison
        nc.clear_and_free_semaphores(list(self.sems.allocated().values()))
        nc.all_engine_barrier()

    tile_mod.TileContext._drain_and_barrier = _drain_and_barrier
    tile_mod._drain_patched = True


def _patch_compiler_wait_split():
    """Pinned walrus accepts only 1 sync wait per instruction encoding slot
    it has available; rewrite the BIR before compiling so every instruction
    carries at most 1 wait, extras moved to preceding same-engine NoOps."""
    import json
    import concourse.bass_utils as bu
    import concourse.bass2jax as b2j

    if getattr(bu, "_wsplit_patched", False):
        return
    orig = bu.compile_bir_kernel

    def fix_block(bb, ctr):
        out = []
        for inst in bb.get("instructions", []):
            for blk in inst.get("blocks") or []:
                fix_block(blk, ctr)
            si = inst.get("sync_info")
            if si:
                ow = si.get("on_wait") or []
                if len(ow) > 1:
                    for w in ow[:-1]:
                        ctr[0] += 1
                        out.append(
                            {
                                "debug": inst.get("debug", 0),
                                "engine": inst["engine"],
                                "ins": [],
                                "name": f"wsplit-{ctr[0]}",
                                "opcode": "NoOp",
                                "outs": [],
                                "text_hint": "wsplit",
                                "sync_info": {"on_wait": [w], "on_update": []},
                            }
                        )
                    si["on_wait"] = [ow[-1]]
            out.append(inst)
        bb["instructions"] = out

    def wrapped(bir_json, tmpdir, neff_name="file.neff"):
        b = json.loads(bir_json)
        ctr = [0]
        for f in b.get("functions", []):
            for bb in f.get("blocks", []):
                fix_block(bb, ctr)
        return orig(json.dumps(b).encode(), tmpdir, neff_name)

    bu.compile_bir_kernel = wrapped
    b2j.compile_bir_kernel = wrapped

    if os.environ.get("LDW_OPT", "0") == "1":
        orig_run = bu.run_command

        def run_patched(argv, **kw):
            argv = [
                "--enable-ldw-opt=true" if a == "--enable-ldw-opt=false" else a
                for a in argv
            ]
            return orig_run(argv, **kw)

        bu.run_command = run_patched
    bu._wsplit_patched = True


def _build(T):
    import concourse.bass as bass
    import concourse.mybir as mybir
    import concourse.tile as tile

    _patch_tile_drain(tile, mybir)
    _patch_compiler_wait_split()
    f32 = mybir.dt.float32
    b16 = mybir.dt.bfloat16
    f16 = mybir.dt.float16
    w8 = mybir.dt.float8e3
    wdt = w8 if _wdt() == "fp8" else b16
    wnp_bytes = 1 if _wdt() == "fp8" else 2
    Sig = mybir.ActivationFunctionType.Sigmoid
    Tanh = mybir.ActivationFunctionType.Tanh
    ADD = mybir.AluOpType.add
    MUL = mybir.AluOpType.mult
    NT = T * BL
    WS = _wscale()

    nc = bass.Bass("TRN2", target_bir_lowering=False)

    x0T = nc.dram_tensor("x0T", [128, NT], b16, kind="ExternalInput")
    mask8 = nc.dram_tensor("mask8", [128, 8 * T], b16, kind="ExternalInput")
    whs, wxs, biases = {}, {}, {}
    for l in range(2):
        nk = 1 if l == 0 else 4
        for d in ("f", "b"):
            whs[(l, d)] = nc.dram_tensor(f"wh{l}{d}", [128, 16 * 128], wdt, kind="ExternalInput")
            wxs[(l, d)] = nc.dram_tensor(f"wx{l}{d}", [128, nk * 8 * 128], b16, kind="ExternalInput")
            biases[(l, d)] = nc.dram_tensor(f"bias{l}{d}", [128, 8], f32, kind="ExternalInput")
    outw = nc.dram_tensor("outw", [128, 20], b16, kind="ExternalInput")
    outb = nc.dram_tensor("outb", [128, 1], f32, kind="ExternalInput")
    identd = nc.dram_tensor("ident", [128, 128], f16, kind="ExternalInput")
    out = nc.dram_tensor("out", [5, NT], f32, kind="ExternalOutput")

    with tile.TileContext(nc) as tc:
        with tc.tile_pool(name="persist", bufs=1) as pp, \
             tc.tile_pool(name="xpbuf", bufs=3) as xpp, \
             tc.tile_pool(name="stage", bufs=3) as sp, \
             tc.tile_pool(name="small", bufs=2) as mp, \
             tc.tile_pool(name="zps", bufs=3, space="PSUM") as zp, \
             tc.tile_pool(name="pps", bufs=2, space="PSUM") as qp, \
             tc.tile_pool(name="xpd", bufs=1, space="DRAM") as dp:

            def load(name, dram, shape, dt):
                t = pp.tile(shape, dt, tag=name, name=name)
                nc.sync.dma_start(t[:], dram[:])
                return t

            x0T_s = load("x0T", x0T, [128, NT], b16)
            mask_s = load("mask8", mask8, [128, 8 * T], b16)
            wh_s = {k: load(f"wh{k[0]}{k[1]}", v, [128, 16 * 128], wdt) for k, v in whs.items()}
            wx_s = {k: load(f"wx{k[0]}{k[1]}", v, list(v.shape), b16) for k, v in wxs.items()}
            bias_s = {k: load(f"bias{k[0]}{k[1]}", v, [128, 8], f32) for k, v in biases.items()}
            outw_s = load("outw", outw, [128, 20], b16)
            outb_s = load("outb", outb, [128, 1], f32)

            # fp16 identity for the xp->PSUM inject matmul
            ident = load("ident", identd, [128, 128], f16)

            hist = {}
            for l in range(2):
                for d in ("f", "b"):
                    hist[(l, d)] = pp.tile([128, (T + 1) * 8], b16, tag=f"hist{l}{d}", name=f"hist{l}{d}")
            for l in range(2):
                nc.vector.memset(hist[(l, "f")][:, 0:8], 0.0)
                nc.vector.memset(hist[(l, "b")][:, T * 8 : T * 8 + 8], 0.0)

            # m-major layout: col = m*(Tseg*4) + t_local*4 + b, so the XP
            # phase's [128,512] per-m tiles land as contiguous DRAM slices
            # (a t-major layout produces 8-byte-strided writes: ~16K
            # descriptors = ~30us per tile, which starved the scans).
            # Segmented 4x per (l,d): finer DRAM write->read dependencies
            # let scan chunks chase the XP phase instead of waiting for the
            # whole projection to finish.
            TSEG = max(T // 4, 128)
            NSEG = T // TSEG
            xp_dram = {
                (l, d): [
                    dp.tile([128, 8 * TSEG * 4], f16, tag=f"xp{l}{d}{g}", name=f"xp{l}{d}{g}")
                    for g in range(NSEG)
                ]
                for l in range(2)
                for d in ("f", "b")
            }

            def hist_rhs(l, d, half, c0):
                """[128, 128t, 4b] output slice of a history for XP1/logits.
                fw output for position t is slot t+1; bw output is slot t."""
                r = hist[(l, d)].rearrange("p (s q) -> p s q", q=8)
                s0 = c0 + 1 if d == "f" else c0
                return r[:, s0 : s0 + 128, half * 4 : half * 4 + 4]

            # poison per m-slot (gate order i,i,j,j -> wait: TF order i,j,f,o
            # => m-slots 0,1=i  2,3=j  4,5=f  6,7=o). Only i and f need
            # poisoning: i -> -POISON (u term ~ 0), f -> +POISON (c carries).
            PVAL = {0: -POISON * WS, 1: -POISON * WS, 4: POISON * WS, 5: POISON * WS}

            def xp_block(l, d, c0):
                nk = 1 if l == 0 else 4
                wx_t = wx_s[(l, d)]
                mask_r = mask_s.rearrange("p (s q) -> p s q", q=8)
                if True:
                    xp_r = xp_dram[(l, d)][c0 // TSEG].rearrange("p (m s) -> p m s", m=8)
                    for m in range(8):
                        ps = qp.tile([128, 512], f32, tag="proj_ps")
                        for k in range(nk):
                            if l == 0:
                                rhs = x0T_s[:, c0 * 4 : c0 * 4 + 512]
                            else:
                                rhs = hist_rhs(0, "f" if k < 2 else "b", k % 2, c0)
                            nc.tensor.matmul(
                                ps[:],
                                wx_t[:, (k * 8 + m) * 128 : (k * 8 + m + 1) * 128],
                                rhs,
                                start=(k == 0),
                                stop=(k == nk - 1),
                            )
                        st = sp.tile([128, 512], f16, tag="xp_st")
                        if d == "b" and m in PVAL:
                            p = PVAL[m]
                            # xp' = (ps + bias - p) * mask + p
                            s1 = sp.tile([128, 512], f32, tag="xp_s1")
                            nc.vector.tensor_scalar_add(s1[:], ps[:], biasP_s[(l, d)][:, m : m + 1])
                            mview = mask_r[:, c0 : c0 + 128, 0:4]
                            nc.vector.tensor_tensor(
                                st.rearrange("p (s q) -> p s q", q=4)[:],
                                s1.rearrange("p (s q) -> p s q", q=4)[:],
                                mview, MUL,
                            )
                            nc.vector.tensor_scalar_add(st[:], st[:], float(p))
                        else:
                            nc.vector.tensor_scalar_add(st[:], ps[:], bias_s[(l, d)][:, m : m + 1])
                        lc = (c0 % TSEG) * 4
                        nc.sync.dma_start(xp_r[:, m, lc : lc + 512], st[:])

            # bias - poison tiles for the poisoned m-slots (per l, bw only)
            biasP_s = {}
            for l in range(2):
                biasP_s[(l, "b")] = pp.tile([128, 8], f32, tag=f"biasP{l}", name=f"biasP{l}")
                bb = biasP_s[(l, "b")]
                nc.vector.tensor_copy(bb[:], bias_s[(l, "b")][:])
                for m, p in PVAL.items():
                    nc.vector.tensor_scalar_add(bb[:, m : m + 1], bb[:, m : m + 1], float(-p))

            def step(l, d, t, xp_tile, i):
                """One LSTM step: z = wh.T@h + xp (PSUM), s = sigmoid(z/WS),
                c = s_f*c + s_i*(2*s_j-1), h = s_o*tanh(c)."""
                h = hist[(l, d)]
                if d == "f":
                    r_off, w_off = t * 8, (t + 1) * 8
                else:
                    r_off, w_off = (t + 1) * 8, t * 8
                z = zp.tile([128, 32], f32, tag=f"z{d}")
                xr = xp_tile.rearrange("p (m s q) -> p m s q", m=8, q=4)
                nc.tensor.matmul(
                    z[:], ident[:], xr[:, :, i, :],
                    start=True, stop=False, skip_group_check=True,
                )
                wh_t = wh_s[(l, d)]
                for k in range(2):
                    for m in range(8):
                        nc.tensor.matmul(
                            z[:, 4 * m : 4 * m + 4],
                            wh_t[:, (k * 8 + m) * 128 : (k * 8 + m + 1) * 128],
                            h[:, r_off + 4 * k : r_off + 4 * k + 4],
                            start=False,
                            stop=(m == 7 and k == 1),
                            skip_group_check=True,
                        )
                s = sp.tile([128, 32], f32, tag=f"s{d}")
                nc.scalar.activation(s[:], z[:], Sig, scale=1.0 / WS)
                c = cs[(l, d)]
                tt = mp.tile([128, 8], f32, tag=f"tt{d}")
                # tt = (s_j - 0.5) * s_i
                nc.vector.scalar_tensor_tensor(tt[:], s[:, 8:16], -0.5, s[:, 0:8], ADD, MUL)
                c1 = mp.tile([128, 8], f32, tag=f"c1{d}")
                nc.vector.tensor_tensor(c1[:], s[:, 16:24], c[:], MUL)
                # c = 2*tt + c1
                nc.vector.scalar_tensor_tensor(c[:], tt[:], 2.0, c1[:], MUL, ADD)
                th = mp.tile([128, 8], f32, tag=f"th{d}")
                nc.scalar.activation(th[:], c[:], Tanh)
                nc.vector.tensor_tensor(h[:, w_off : w_off + 8], s[:, 24:32], th[:], MUL)

            cs = {}
            for l in range(2):
                for d in ("f", "b"):
                    cs[(l, d)] = pp.tile([128, 8], f32, tag=f"c{l}{d}", name=f"c{l}{d}")

            for l in range(2):
                # interleave fw (ascending) and bw (descending) projection
                # blocks so BOTH scans' first chunks exist after ~2 blocks
                for c0 in range(0, T, 128):
                    xp_block(l, "f", c0)
                    xp_block(l, "b", T - 128 - c0)
                nc.vector.memset(cs[(l, "f")][:], 0.0)
                nc.vector.memset(cs[(l, "b")][:], 0.0)
                for c0 in range(0, T, CH):
                    # chunk tiles are m-major: [128, m(8), CH*4]
                    xf = xpp.tile([128, 8 * CH * 4], f16, tag="xf")
                    src_f = xp_dram[(l, "f")][c0 // TSEG].rearrange("p (m s) -> p m s", m=8)
                    lf = (c0 % TSEG) * 4
                    nc.sync.dma_start(
                        xf.rearrange("p (m s) -> p m s", m=8)[:],
                        src_f[:, :, lf : lf + CH * 4],
                    )
                    xb = xpp.tile([128, 8 * CH * 4], f16, tag="xb")
                    bw_lo = T - c0 - CH
                    src_b = xp_dram[(l, "b")][bw_lo // TSEG].rearrange("p (m s) -> p m s", m=8)
                    lb = (bw_lo % TSEG) * 4
                    nc.sync.dma_start(
                        xb.rearrange("p (m s) -> p m s", m=8)[:],
                        src_b[:, :, lb : lb + CH * 4],
                    )
                    for i in range(CH):
                        step(l, "f", c0 + i, xf, i)
                        step(l, "b", T - 1 - (c0 + i), xb, CH - 1 - i)

            # ---- logits ----
            mask_r = mask_s.rearrange("p (s q) -> p s q", q=8)
            for c0 in range(0, T, 128):
                ps = qp.tile([128, 512], f32, tag="proj_ps")
                for k in range(4):
                    rhs = hist_rhs(1, "f" if k < 2 else "b", k % 2, c0)
                    nc.tensor.matmul(
                        ps[:5, :],
                        outw_s[:, k * 5 : k * 5 + 5],
                        rhs,
                        start=(k == 0),
                        stop=(k == 3),
                    )
                lg = sp.tile([5, 512], f32, tag="lg")
                nc.vector.tensor_mul(lg[:], ps[:5, :], mask_r[:5, c0 : c0 + 128, 0:4])
                nc.vector.tensor_scalar_add(lg[:], lg[:], outb_s[:5, 0:1])
                nc.sync.dma_start(out[:, c0 * 4 : c0 * 4 + 512], lg[:])

    return nc


last_results = None


def kernel(**inputs):
    global last_results
    T = int(os.environ.get("KERNEL_T", T_FULL))
    from concourse.bass_utils import run_bass_kernel_spmd

    tokens = np.asarray(inputs["tokens"])[:, :T]
    lengths = np.clip(np.asarray(inputs["lengths"]), 0, T)
    emb = np.asarray(inputs["emb"], dtype=np.float32)
    WS = _wscale()
    w8np = ml_dtypes.float8_e3m4

    if T not in _cache:
        _cache[T] = _build(T)
    nc = _cache[T]

    # ---- host-side retiling (shared across cores) ----
    shared = {}
    for l in range(2):
        D = EMB if l == 0 else 2 * HID
        nk = D // 128
        for d, pre in (("f", "fw"), ("b", "bw")):
            W = np.asarray(inputs[f"{pre}_W{l}"], dtype=np.float32).copy()
            bias = np.asarray(inputs[f"{pre}_b{l}"], dtype=np.float32).copy()
            bias[2 * HID : 3 * HID] += FORGET_BIAS
            # double the j-gate columns: sigmoid(2*z_j) = (tanh(z_j)+1)/2
            W[:, HID : 2 * HID] *= 2.0
            bias[HID : 2 * HID] *= 2.0
            wh = _tile_lhsT(W[D:] * WS, 2, 8)
            if _wdt() == "fp8":
                shared[f"wh{l}{d}"] = wh.astype(w8np)
            else:
                shared[f"wh{l}{d}"] = wh.astype(bf16)
            shared[f"wx{l}{d}"] = _tile_lhsT(W[:D] * WS, nk, 8).astype(bf16)
            shared[f"bias{l}{d}"] = np.ascontiguousarray(
                (bias * WS).reshape(8, 128).T
            ).astype(np.float32)
    shared["outw"] = np.ascontiguousarray(
        np.asarray(inputs["out_W"], dtype=np.float32)
        .reshape(4, 128, 5)
        .transpose(1, 0, 2)
        .reshape(128, 20)
    ).astype(bf16)
    ob = np.zeros((128, 1), np.float32)
    ob[:5, 0] = np.asarray(inputs["out_b"], dtype=np.float32)
    shared["outb"] = ob
    shared["ident"] = np.eye(128, dtype=np.float16)

    in_maps = []
    for ci in range(NCORES):
        bs = slice(ci * BL, (ci + 1) * BL)
        x0 = emb[tokens[bs]]  # [BL, T, 128]
        x0T = np.ascontiguousarray(x0.transpose(2, 1, 0).reshape(128, T * BL)).astype(bf16)
        mvec = (np.arange(T)[:, None] < lengths[bs][None, :]).astype(bf16)  # [T, BL]
        m8 = np.repeat(mvec[:, None, :], 2, axis=1).reshape(1, T * 8)
        mask8 = np.ascontiguousarray(np.broadcast_to(m8, (128, T * 8)))
        im = dict(shared)
        im["x0T"] = x0T
        im["mask8"] = mask8
        in_maps.append(im)

    res = run_bass_kernel_spmd(nc, in_maps, core_ids=list(range(NCORES)))
    last_results = res
    outs = []
    for ci in range(NCORES):
        o = res.results[ci]["out"]  # [5, T*BL]
        outs.append(o.reshape(5, T, BL).transpose(2, 1, 0))  # [BL, T, 5]
    return np.concatenate(outs, axis=0).astype(np.float32)
